# revision 30
# baseline (speedup 1.0000x reference)
"""Entropy-regularized attention (standard MHA fwd) on 8 trn2 cores.

Sharding: core c -> batch b=c//4, head-group g=c%4 (4 of 16 heads).
Each core computes q/k/v for its 256-wide head-group slice, transposed-
layout attention, then a row-split Wo partial product. Host sums the 4
partials per batch and adds bo (the "all-reduce").

v3 restructure vs v2 (261us -> 227us measured):
- Scores are emitted as ROW-TILED HEAD PAIRS: the two heads of an mt
  group have K=64 stationaries at base partitions 0 and 64, so their
  score matmuls auto-derive tile_position (0,0)/(64,0) and execute
  CONCURRENTLY on the PE (each K=64 tile uses half the array rows).
  This halves the PE time of the scores phase (PE busy 231 -> ~191us).
- QG=512 with 4 query groups: per pair-unit the scores for both heads
  land in one [128,2,512] half of the scM PSUM tile and ONE Exp
  (N=1024) covers the pair. PSUM: scM 4 banks + av 2 + ops 2 = 8.
- Softmax 1/l: per qg ONE batched Ln + ONE Exp(-x) on ACT ([97,512]
  costs the same as [1,512]), GPSIMD partition_broadcast fans each
  head's row across 64 partitions (input must sit at PHYSICAL
  partition 0 - it ignores the AP base), DVE tensor_mul into avT.
  The Ln/Exp+muls are release-delayed ~4 units so they queue in ACT's
  FIFO behind already-ready exps. The last qg's second pair instead
  runs per-head Ln/Exp straight from PSUM + PE rb-matmul broadcast to
  minimize the tail after the final exp.
- ALL non-score PE work lives in deferred queues popped under a
  per-unit cost budget (~880ns). av matmuls are release-delayed 3
  units: the PE queue is STRICT FIFO, so an av emitted next to its
  exp blocks later score matmuls while waiting; 3 units late their
  exp has long completed. Readiness gates (qk_ready/v_done +
  ensure_av_through) force-pop so no consumer is ever EMITTED before
  its producer (the Tile framework only orders writer->reader when
  the writer is emitted first; violating that is a silent HW race).
- KNOWN FLOOR: the unit cadence is ~1736ns (vs exp's 1114): with only
  2 scM halves, sc(u) truly depends on exp(u-2), but the Tile
  framework lowers cross-engine deps to engine-frontier counter
  values ("wait ACT-count >= last-emitted-exp"), so whichever of
  sc/exp is emitted later picks up a false one-unit-back edge; every
  emission order tried (fused, split, lookahead-2) leaves one such
  edge on the critical cycle: cadence = exp 1114 + sc chain ~620.
  Breaking it needs >=3 score buffers (PSUM-infeasible at QG=512) or
  raw-bass manual semaphores. TILE_EXHAUSTIVE_MEMORY_SHARE_CHECK=1 is
  set (precise range deps; needed for the manual scM half rotation).
"""

import os
import sys

# Precise (range/bank-exact) memory dependency analysis: without this the
# Tile dep tracker caps its overlap-check work and falls back to
# engine-frontier edges, which serialize the exp stream with the score
# matmuls' PSUM-slot reuse (~600ns per attention unit).
os.environ.setdefault("TILE_EXHAUSTIVE_MEMORY_SHARE_CHECK", "1")

for _p in ("/opt/trn_rl_repo", "/root/.axon_site/_ro/trn_rl_repo"):
    if _p not in sys.path:
        sys.path.insert(0, _p)

import numpy as np

import concourse.bass as bass
import concourse.mybir as mybir
import concourse.tile as tile
from concourse import bacc

P = 128
S = 2048  # sequence length
D = 1024  # hidden
DG = 256  # per-core head-group width (4 heads x 64)
HD = 64
NHL = 4  # heads per core
KT_D = D // P  # 8 contraction tiles for projections
ST = S // P  # 16 sequence tiles
QG = 512  # query-group width
NQG = S // QG  # 4 query groups
NPR = 2  # head pairs per core

F32 = mybir.dt.float32
F32R = mybir.dt.float32r
BF16 = mybir.dt.bfloat16


def build_nc():
    nc = bacc.Bacc(None, target_bir_lowering=False)

    # all inputs pre-arranged on the host into the exact SBUF layouts so
    # every DMA is a contiguous-per-partition blob (large descriptors)
    xT = nc.dram_tensor("xT", [P, 4 * KT_D * 512], BF16, kind="ExternalInput")
    wq = nc.dram_tensor("wq", [P, KT_D * DG], BF16, kind="ExternalInput")
    wk = nc.dram_tensor("wk", [P, KT_D * DG], BF16, kind="ExternalInput")
    wv = nc.dram_tensor("wv", [P, KT_D * DG], BF16, kind="ExternalInput")
    wo = nc.dram_tensor("wo", [P, 2 * D], BF16, kind="ExternalInput")
    bq = nc.dram_tensor("bq", [P, 2], F32, kind="ExternalInput")
    bk = nc.dram_tensor("bk", [P, 2], F32, kind="ExternalInput")
    bv = nc.dram_tensor("bv", [1, DG], F32R, kind="ExternalInput")
    out = nc.dram_tensor("out", [S, D], BF16, kind="ExternalOutput")

    with tile.TileContext(nc) as tc:
        _body(tc, nc, xT, wq, wk, wv, wo, bq, bk, bv, out)

    # Pin Exp/Ln to the one table set holding both: strip them from the
    # competing sets (dict order and size preserved, so act_func_set_id
    # indices stay valid). Without this the table-load pass can bounce
    # between table sets.
    import concourse.bacc as _bacc_mod

    _orig_tables = _bacc_mod.get_activation_tables

    def _pinned_tables(arch):
        t = _orig_tables(arch)
        for name, fns in t.items():
            if name != "natural_log_exp_and_others":
                fns.discard(mybir.ActivationFunctionType.Exp)
                fns.discard(mybir.ActivationFunctionType.Ln)
        return t

    _bacc_mod.get_activation_tables = _pinned_tables
    try:
        nc.compile()
    finally:
        _bacc_mod.get_activation_tables = _orig_tables
    return nc


def _body(tc, nc, xT, wq, wk, wv, wo, bq, bk, bv, out):
    from collections import deque
    from contextlib import ExitStack

    with ExitStack() as ctx:
        ctx.enter_context(
            nc.allow_low_precision(
                reason="bf16 matmul inputs; accumulation is fp32 PSUM"
            )
        )
        persist = ctx.enter_context(tc.tile_pool(name="persist", bufs=1))
        expool = ctx.enter_context(tc.tile_pool(name="expool", bufs=16))
        ulpool = ctx.enter_context(tc.tile_pool(name="ulpool", bufs=12))
        rpool = ctx.enter_context(tc.tile_pool(name="rpool", bufs=4))
        l4pool = ctx.enter_context(tc.tile_pool(name="l4pool", bufs=4))
        r4pool = ctx.enter_context(tc.tile_pool(name="r4pool", bufs=2))
        opool = ctx.enter_context(tc.tile_pool(name="opool", bufs=3))
        # PSUM budget (8 banks): scM [128,4,512]=4 (manually rotated
        # halves), avM [65,2,512]=2, ops 2x[128,512]=2. scM/avM are ONE
        # persistent tile each: pool slot rotation pins a conservative
        # WAR edge to the ACT frontier at emission time (costing ~600ns
        # per unit); manual ranges get precise subtile dependencies.
        ps_persist = ctx.enter_context(
            tc.tile_pool(name="ps_persist", bufs=1, space="PSUM")
        )
        ps_av = ctx.enter_context(tc.tile_pool(name="ps_av", bufs=2, space="PSUM"))
        ps_o = ctx.enter_context(tc.tile_pool(name="ps_o", bufs=2, space="PSUM"))

        qT_sb = persist.tile([P, 2, S], BF16)
        kT_sb = persist.tile([P, 2, S], BF16)
        v_sb = persist.tile([P, ST, NHL * 65], BF16)  # 65-striped: col 64 = ones
        avT = [
            persist.tile([P, 2, QG], BF16, tag=f"avT{g}", name=f"avT{g}")
            for g in range(NQG)
        ]
        wo_sb = persist.tile([P, 2, D], BF16)
        ones_row = persist.tile([1, P], F32R)
        xT_sb = persist.tile([P, 4, KT_D, 512], BF16)
        wq_sb = persist.tile([P, KT_D, DG], BF16, tag="wq")
        wk_sb = persist.tile([P, KT_D, DG], BF16, tag="wk")
        wv_sb = persist.tile([P, KT_D, DG], BF16, tag="wv")
        bq_sb = persist.tile([P, 2], F32, tag="bq")
        bk_sb = persist.tile([P, 2], F32, tag="bk")
        bv_sb = persist.tile([1, DG], F32R, tag="bv")

        # DMAs ordered to match first-use: wk then the first xT column
        # block gate the first projection chain.
        xTr = xT.rearrange("p (cb kt s) -> p cb kt s", kt=KT_D, s=512)
        nc.sync.dma_start(wk_sb[:], wk.rearrange("p (kt n) -> p kt n", n=DG))
        nc.sync.dma_start(xT_sb[:, 0, 0:2], xTr[:, 0, 0:2])
        nc.sync.dma_start(xT_sb[:, 0, 2:4], xTr[:, 0, 2:4])
        nc.sync.dma_start(bk_sb[:], bk[:])
        nc.sync.dma_start(wq_sb[:], wq.rearrange("p (kt n) -> p kt n", n=DG))
        nc.sync.dma_start(xT_sb[:, 0, 4:6], xTr[:, 0, 4:6])
        nc.sync.dma_start(xT_sb[:, 0, 6:8], xTr[:, 0, 6:8])
        nc.sync.dma_start(bq_sb[:], bq[:])
        nc.sync.dma_start(wv_sb[:], wv.rearrange("p (kt n) -> p kt n", n=DG))
        nc.sync.dma_start(bv_sb[:], bv[:])
        nc.sync.dma_start(xT_sb[:, 1], xTr[:, 1])
        nc.sync.dma_start(xT_sb[:, 2], xTr[:, 2])
        nc.sync.dma_start(xT_sb[:, 3], xTr[:, 3])
        nc.sync.dma_start(wo_sb[:], wo.rearrange("p (kt n) -> p kt n", n=D))

        # memset can't emit float32r; stage fp32 ones and copy-cast
        ones_f32 = persist.tile([P, P], F32)
        nc.vector.memset(ones_f32[:], 1.0)
        nc.vector.tensor_copy(ones_row[:], ones_f32[0:1, :])
        ones_all = persist.tile([P, P], BF16)
        nc.vector.tensor_copy(ones_all[:], ones_f32[:])
        nc.vector.tensor_copy(
            v_sb.rearrange("p st (h w) -> p st h w", w=65)[:, :, :, 64],
            ones_f32[:, 0:64].rearrange("p (st h) -> p st h", h=4),
        )

        # ---- projection building blocks ----
        def qk_half(ps, wsb, mt, nq, kts):
            for kt in kts:
                nc.tensor.matmul(
                    ps[:, 0:512],
                    wsb[:, kt, mt * P : (mt + 1) * P],
                    xT_sb[:, nq, kt, :],
                    start=(kt == 0),
                    stop=(kt == KT_D - 1),
                )

        def qk_evict(ps, bsb, dest, mt, nq):
            nc.vector.tensor_scalar_add(
                dest[:, mt, nq * 512 : (nq + 1) * 512],
                ps[:, 0:512],
                bsb[:, mt : mt + 1],
            )

        # emission-order readiness flags: a score matmul may only be
        # EMITTED after the projection chain writing its qT/kT region has
        # been emitted (the Tile framework tracks writer->reader deps in
        # program order; a later-emitted writer would be a race).
        qk_ready = {}  # ("q"/"k", mt, nq) -> True

        def qk_full(wsb, bsb, dest, mt, nq, key):
            ps = ps_o.tile([P, 512], F32, tag="ops", name="qkps")
            qk_half(ps, wsb, mt, nq, range(0, KT_D))
            qk_evict(ps, bsb, dest, mt, nq)
            qk_ready[(key, mt, nq)] = True

        def qk_pieces(wsb, bsb, dest, mt, nq, key, pieces=4):
            # split the 8-matmul K-chain into `pieces` closures; the last
            # one carries the bias eviction. cost ~= (8/pieces)*213ns
            box = []
            step = KT_D // pieces

            def make(pi):
                def run():
                    if pi == 0:
                        box.append(
                            ps_o.tile([P, 512], F32, tag="ops", name="qkps")
                        )
                    ps = box[-1]
                    qk_half(ps, wsb, mt, nq, range(pi * step, (pi + 1) * step))
                    if pi == pieces - 1:
                        qk_evict(ps, bsb, dest, mt, nq)
                        box.pop()
                        qk_ready[(key, mt, nq)] = True

                return run

            return [(step * 215, make(pi)) for pi in range(pieces)]

        v_done = [False] * ST

        def v_group_half(st, half):
            # half 0: kts 0-3; half 1: kts 4-7 + bias + evict
            def run():
                if half == 0:
                    v_group_half.box[st] = ps_o.tile(
                        [P, 512], F32, tag="ops", name="vps"
                    )
                    ps = v_group_half.box[st]
                    for kt in range(0, 4):
                        nc.tensor.matmul(
                            ps[:, 0:DG],
                            xT_sb[:, st // 4, kt, (st % 4) * P : (st % 4 + 1) * P],
                            wv_sb[:, kt, :],
                            start=(kt == 0),
                            stop=False,
                        )
                else:
                    ps = v_group_half.box.pop(st)
                    for kt in range(4, KT_D):
                        nc.tensor.matmul(
                            ps[:, 0:DG],
                            xT_sb[:, st // 4, kt, (st % 4) * P : (st % 4 + 1) * P],
                            wv_sb[:, kt, :],
                            start=False,
                            stop=False,
                        )
                    nc.tensor.matmul(
                        ps[:, 0:DG],
                        ones_row[0:1, 0:P],
                        bv_sb[0:1, :],
                        start=False,
                        stop=True,
                    )
                    nc.vector.tensor_copy(
                        v_sb.rearrange("p st (h w) -> p st h w", w=65)[
                            :, st, :, 0:64
                        ],
                        ps[:, 0:DG].rearrange("p (h w) -> p h w", w=64),
                    )
                    v_done[st] = True

            return run

        v_group_half.box = {}

        def v_group_full(st):
            v_group_half(st, 0)()
            v_group_half(st, 1)()

        # ---- deferred machinery ----
        # The PE engine queue is strict FIFO: an emitted matmul that waits
        # on a semaphore blocks everything emitted after it. So av pieces
        # (which wait on their exp) are RELEASE-DELAYED ~3 units: by the
        # time they enter the queue their exp has long finished, and the
        # next units' score matmuls are never stuck behind them. `delayed`
        # is strictly FIFO (entries release in order once their
        # release-iter arrives and their v-group gate is satisfied).
        # avq: released attention follow-up (av/norms), popped first.
        # defq: projection / Wo filler under a per-unit cost budget.
        avq = deque()
        defq = deque()
        delayed = deque()  # (release_iter, gate_st_or_None, kind, cost, fn)

        BUDGET = 880
        n_av_emitted = [0]  # count of av matmul pieces emitted (2 per unit)

        def release_delayed(i):
            while delayed and delayed[0][0] <= i:
                _, gate, kind, cost, fn = delayed[0]
                if gate is not None and not v_done[gate]:
                    break
                delayed.popleft()
                avq.append((kind, cost, fn))

        def pop_one_av():
            if avq:
                kind, _, fn = avq.popleft()
                fn()
                if kind == "av":
                    n_av_emitted[0] += 1
                return True
            return False

        def pop_one_def():
            if defq:
                _, fn = defq.popleft()
                fn()
                return True
            return False

        def pop_pieces(i):
            spent = 0
            release_delayed(i)
            while avq and spent < BUDGET:
                kind, cost, fn = avq.popleft()
                fn()
                if kind == "av":
                    n_av_emitted[0] += 1
                spent += cost
            while defq and spent < BUDGET:
                cost, fn = defq.popleft()
                fn()
                spent += cost

        def ensure_qk(key, mt, nq):
            # force-pop until the projection chain for this region has run
            while not qk_ready.get((key, mt, nq)):
                if not pop_one_def():
                    raise RuntimeError(f"deadlock: {key} mt{mt} nq{nq}")

        def ensure_av_through(unit_idx):
            # all av pieces of units <= unit_idx emitted (ex-slot reuse)
            while n_av_emitted[0] < 2 * (unit_idx + 1):
                if pop_one_av():
                    continue
                if delayed:
                    _, gate, kind, cost, fn = delayed[0]
                    if gate is None or v_done[gate]:
                        delayed.popleft()
                        avq.append((kind, cost, fn))
                        continue
                if not pop_one_def():
                    raise RuntimeError("deadlock: av drain")

        # ---- attention phase ----
        EXBUFS = 16  # must match expool bufs

        scM = ps_persist.tile([P, 4, 512], F32, tag="scM", name="scM")

        def sc_emit(u):
            # score pair for unit u into scM half u%2. Emitted BEFORE
            # exp(u-1) (see the unit loop), so the conservative
            # PSUM-write-after-ACT-read frontier edge the framework pins
            # on this matmul is exp(u-2) -- exactly the true dependency
            # (exp(u-2) is the reader of this half). The pair then
            # pre-executes during exp(u-1) and never gates the exp stream.
            pr, qg, kt = units[u]
            ensure_qk("k", pr, kt // 4)
            ensure_qk("q", pr, qg)
            mt, q0 = pr, qg * QG
            half = (u % 2) * 2
            for hh in range(2):
                po = hh * 64
                nc.tensor.matmul(
                    scM[:, half + hh, :],
                    kT_sb[po : po + 64, mt, kt * P : (kt + 1) * P],
                    qT_sb[po : po + 64, mt, q0 : q0 + 512],
                    start=True,
                    stop=True,
                )

        def exp_emit(u):
            half = (u % 2) * 2
            ex = expool.tile([P, 2, 512], BF16, tag="ex", name="ex")
            nc.scalar.activation(
                ex[:], scM[:, half : half + 2, :],
                mybir.ActivationFunctionType.Exp, scale=0.125,
            )
            return ex

        avps = {}  # hh -> live av psum tile for current (pr, qg)

        def av_piece(pr, qg, kt, hh, ex):
            h = 2 * pr + hh

            def run():
                if kt == 0:
                    avps[hh] = ps_av.tile([65, 512], F32, tag="av", name="av")
                nc.tensor.matmul(
                    avps[hh][0:65, :],
                    v_sb[:, kt, h * 65 : h * 65 + 65],
                    ex[:, hh, :],
                    start=(kt == 0),
                    stop=(kt == ST - 1),
                )

            return (215, run)

        # Softmax normalization: per (pr,qg,head) the av PSUM is copied to
        # SBUF (ul) and its denominator row gathered into l4[qg] at
        # partition 32h. Once a qg's heads are all gathered, ONE batched
        # Ln + ONE Exp(-x) on ACT produce 1/l ([97,512] costs the same as
        # [1,512]); GPSIMD broadcasts each head's row across 64 partitions
        # and DVE multiplies into avT. The Ln/Exp+muls are DELAYED ~4
        # units so they land in ACT's FIFO behind already-ready exps
        # (emitting them immediately stalls the exp stream on the av->ul->
        # gather dependency chain). The LAST qg's second pair instead runs
        # per-head Ln/Exp straight from PSUM with a PE-matmul broadcast,
        # minimizing the serial tail after the final exp.
        uls = {}
        l4s = {}
        r4box = {}

        def p_ul_make(pr, qg, hh, skip_l=False):
            h = 2 * pr + hh

            def p_ul():
                av = avps.pop(hh)
                if qg not in l4s:
                    l4s[qg] = l4pool.tile([97, 512], F32, tag="l4", name="l4")
                    nc.gpsimd.memset(l4s[qg][:], 1.0)
                ul = ulpool.tile([65, 512], F32, tag="ul", name="ul")
                nc.vector.tensor_copy(ul[:], av[0:65, :])
                if not skip_l:
                    nc.vector.tensor_copy(
                        l4s[qg][32 * h : 32 * h + 1, :], av[64:65, :]
                    )
                uls[(qg, h)] = ul

            return p_ul

        def p_norm_make(qg, hi):
            def p_norm():
                # 1/l = exp(-ln(l)) for heads 0..hi/32 in one Ln + one Exp
                l4 = l4s.pop(qg)
                ls = r4pool.tile([97, 512], F32, tag="ls", name="ls")
                nc.scalar.activation(
                    ls[0:hi], l4[0:hi], mybir.ActivationFunctionType.Ln
                )
                r4 = r4pool.tile([97, 512], F32, tag="r4", name="r4")
                nc.scalar.activation(
                    r4[0:hi], ls[0:hi],
                    mybir.ActivationFunctionType.Exp, scale=-1.0,
                )
                r4box[qg] = r4

            return p_norm

        def p_mul_make(qg, h2, unlock_wo=False):
            def p_mul():
                mt2, po2 = h2 // 2, (h2 % 2) * 64
                ul = uls.pop((qg, h2))
                # partition_broadcast reads physical partition 0: stage
                # this head's reciprocal row to a base-0 tile first
                r1 = rpool.tile([1, 512], F32, tag="r1", name="r1")
                nc.vector.tensor_copy(
                    r1[:], r4box[qg][32 * h2 : 32 * h2 + 1, :]
                )
                rbb = rpool.tile([64, 512], F32, tag="rbb", name="rbb")
                nc.gpsimd.partition_broadcast(rbb[:], r1[:], channels=64)
                nc.vector.tensor_mul(
                    out=avT[qg][po2 : po2 + 64, mt2, :],
                    in0=ul[0:64, :],
                    in1=rbb[:],
                )
                if unlock_wo:
                    defq.extend(wo_pieces(qg))

            return p_mul

        def tail_head_pieces(qg, hh):
            # last qg, second pair: Ln/Exp straight from the av PSUM, PE
            # rb-matmul broadcast (PE is idle in the tail), DVE multiply.
            h = 2 + hh
            box = {}

            def p_lnexp():
                av = avps[hh]
                lnr = rpool.tile([1, 512], F32, tag="lnr", name="lnr")
                nc.scalar.activation(
                    lnr[:], av[64:65, :], mybir.ActivationFunctionType.Ln
                )
                r1 = rpool.tile([1, 512], BF16, tag="r1b", name="r1b")
                nc.scalar.activation(
                    r1[:], lnr[:], mybir.ActivationFunctionType.Exp,
                    scale=-1.0,
                )
                box["r1"] = r1

            def p_ul():
                av = avps.pop(hh)
                ul = ulpool.tile([65, 512], F32, tag="ul", name="ul")
                nc.vector.tensor_copy(ul[0:64, :], av[0:64, :])
                box["ul"] = ul

            def p_rbmul():
                rb = ps_o.tile([P, 512], F32, tag="ops", name="rb")
                nc.tensor.matmul(
                    rb[0:64, :],
                    ones_all[0:1, 0:64],
                    box["r1"][:],
                    start=True,
                    stop=True,
                )
                nc.vector.tensor_mul(
                    out=avT[qg][hh * 64 : hh * 64 + 64, 1, :],
                    in0=box["ul"][0:64, :],
                    in1=rb[0:64, :],
                )
                if hh == 1:
                    defq.extend(wo_pieces(qg))

            return [(50, p_lnexp), (50, p_ul), (250, p_rbmul)]

        ot_box = {}

        def wo_pieces(qg):
            # per (sti, nd): 2 matmuls (kt2 accumulation) + eviction; the
            # ops-pool slot is alloc'd and freed within one piece so the
            # bufs=2 rotation can interleave with qk/v pieces.
            pieces = []

            def make(sti, nd):
                def run():
                    st = qg * (QG // P) + sti
                    if nd == 0:
                        ot_box[sti] = opool.tile([P, D], BF16, tag="ot", name="ot")
                    ot = ot_box[sti]
                    pp = ps_o.tile([P, 512], F32, tag="ops", name="pp")
                    for kt2 in range(2):
                        nc.tensor.matmul(
                            pp[:],
                            avT[qg][:, kt2, sti * P : (sti + 1) * P],
                            wo_sb[:, kt2, nd * 512 : (nd + 1) * 512],
                            start=(kt2 == 0),
                            stop=(kt2 == 1),
                        )
                    nc.vector.tensor_copy(ot[:, nd * 512 : (nd + 1) * 512], pp[:])
                    if nd == 1:
                        del ot_box[sti]
                        nc.sync.dma_start(out[st * P : (st + 1) * P, :], ot[:])

                return run

            for sti in range(QG // P):
                for nd in range(2):
                    pieces.append((460, make(sti, nd)))
            return pieces

        # ---- upfront phase: only what the FIRST sc/exp needs (k+q mt0
        # nq0); v groups are deferred (av lags behind the exp stream) ----
        qk_full(wk_sb, bk_sb, kT_sb, 0, 0, "k")
        qk_full(wq_sb, bq_sb, qT_sb, 0, 0, "q")

        # ---- static filler: ordered by need-by unit ----
        # kT mt0 fully by unit ~12 (sc consumes kt blocks 4/8/12 at units
        # 4/8/12); q(mt0,nq1..3) by units 16/32/48; v(st) before av(st)
        # emission (forced by ensure_av_through); mt1 chains by unit 64.
        for st in (0, 1):
            defq.append((500, v_group_half(st, 0)))
            defq.append((560, v_group_half(st, 1)))
        defq.extend(qk_pieces(wk_sb, bk_sb, kT_sb, 0, 1, "k"))
        defq.extend(qk_pieces(wk_sb, bk_sb, kT_sb, 0, 2, "k"))
        defq.extend(qk_pieces(wk_sb, bk_sb, kT_sb, 0, 3, "k"))
        defq.extend(qk_pieces(wq_sb, bq_sb, qT_sb, 0, 1, "q"))
        for st in (2, 3, 4, 5):
            defq.append((500, v_group_half(st, 0)))
            defq.append((560, v_group_half(st, 1)))
        defq.extend(qk_pieces(wq_sb, bq_sb, qT_sb, 0, 2, "q"))
        for st in (6, 7, 8, 9):
            defq.append((500, v_group_half(st, 0)))
            defq.append((560, v_group_half(st, 1)))
        defq.extend(qk_pieces(wq_sb, bq_sb, qT_sb, 0, 3, "q"))
        for st in (10, 11, 12, 13, 14, 15):
            defq.append((500, v_group_half(st, 0)))
            defq.append((560, v_group_half(st, 1)))
        for nq in range(4):
            defq.extend(qk_pieces(wk_sb, bk_sb, kT_sb, 1, nq, "k"))
            defq.extend(qk_pieces(wq_sb, bq_sb, qT_sb, 1, nq, "q"))

        # ---- the unit loop ----
        units = [
            (pr, qg, kt)
            for pr in range(NPR)
            for qg in range(NQG)
            for kt in range(ST)
        ]
        sc_emit(0)
        sc_emit(1)
        for i, (pr, qg, kt) in enumerate(units):
            ensure_av_through(i - EXBUFS)
            ex = exp_emit(i)
            if i + 2 < len(units):
                sc_emit(i + 2)
            for hh in range(2):
                delayed.append((i + 6, kt, "av", *av_piece(pr, qg, kt, hh, ex)))
            if kt == ST - 1:
                last_qg = qg == NQG - 1
                if pr == 0:
                    for hh in range(2):
                        delayed.append(
                            (i + 4, None, "norm", 50, p_ul_make(pr, qg, hh))
                        )
                    if last_qg:
                        # qg3 pair-0 heads: batched 2-head norm
                        delayed.append(
                            (i + 5, None, "norm", 50, p_norm_make(qg, 33))
                        )
                        delayed.append(
                            (i + 5, None, "norm", 50, p_mul_make(qg, 0))
                        )
                        delayed.append(
                            (i + 5, None, "norm", 50, p_mul_make(qg, 1))
                        )
                elif not last_qg:
                    for hh in range(2):
                        delayed.append(
                            (i + 4, None, "norm", 50, p_ul_make(pr, qg, hh))
                        )
                    delayed.append(
                        (i + 5, None, "norm", 50, p_norm_make(qg, 97))
                    )
                    for h2 in range(4):
                        delayed.append(
                            (i + 5, None, "norm", 50,
                             p_mul_make(qg, h2, unlock_wo=(h2 == 3)))
                        )
                else:
                    # last unit: minimal-latency tail for heads 2,3
                    t0 = tail_head_pieces(qg, 0)
                    t1 = tail_head_pieces(qg, 1)
                    for piece in (t0[0], t1[0], t0[1], t1[1], t0[2], t1[2]):
                        delayed.append((i, None, "norm", *piece))
            pop_pieces(i)
        # ---- tail: drain everything ----
        while delayed or avq or defq:
            release_delayed(10 ** 9)
            if pop_one_av():
                continue
            if not pop_one_def():
                if delayed:
                    raise RuntimeError("tail deadlock")


_NC_CACHE = None


def get_nc():
    global _NC_CACHE
    if _NC_CACHE is None:
        _NC_CACHE = build_nc()
    return _NC_CACHE


def make_in_maps(x, Wq, bq, Wk, bk, Wv, bv, Wo, bo):
    import ml_dtypes

    bf16 = ml_dtypes.bfloat16

    def w_arr(W, sl):
        # [D, DG] -> [p, kt*DG]: W[kt*128+p, n] at [p, kt, n]
        return np.ascontiguousarray(
            W[:, sl].reshape(KT_D, P, DG).transpose(1, 0, 2).reshape(P, -1)
        ).astype(bf16)

    in_maps = []
    for c in range(8):
        b, g = c // 4, c % 4
        sl = slice(g * DG, (g + 1) * DG)
        # x[b].T is [D, S]; SBUF wants [p, cb, kt, 512] with row kt*128+p,
        # col cb*512+s
        xt = (
            x[b]
            .T.reshape(KT_D, P, 4, 512)
            .transpose(1, 2, 0, 3)
            .reshape(P, -1)
            .astype(bf16)
        )
        wo_a = np.ascontiguousarray(
            Wo[sl, :].reshape(2, P, D).transpose(1, 0, 2).reshape(P, -1)
        ).astype(bf16)
        in_maps.append(
            {
                "xT": np.ascontiguousarray(xt),
                "wq": w_arr(Wq, sl),
                "wk": w_arr(Wk, sl),
                "wv": w_arr(Wv, sl),
                "wo": wo_a,
                "bq": np.ascontiguousarray(bq[sl].reshape(2, P).T),
                "bk": np.ascontiguousarray(bk[sl].reshape(2, P).T),
                "bv": np.ascontiguousarray(bv[sl].reshape(1, DG)),
            }
        )
    return in_maps


def kernel(x, Wq, bq, Wk, bk, Wv, bv, Wo, bo, _run_kwargs=None):
    from concourse.bass_utils import run_bass_kernel_spmd

    x = np.asarray(x, dtype=np.float32)
    nc = get_nc()
    in_maps = make_in_maps(
        x,
        np.asarray(Wq, np.float32),
        np.asarray(bq, np.float32),
        np.asarray(Wk, np.float32),
        np.asarray(bk, np.float32),
        np.asarray(Wv, np.float32),
        np.asarray(bv, np.float32),
        np.asarray(Wo, np.float32),
        np.asarray(bo, np.float32),
    )
    res = run_bass_kernel_spmd(
        nc, in_maps, core_ids=list(range(8)), **(_run_kwargs or {})
    )
    bo = np.asarray(bo, np.float32)
    outp = np.empty((2, S, D), dtype=np.float32)
    for b in range(2):
        acc = res.results[4 * b]["out"].astype(np.float32)
        for g in range(1, 4):
            acc = acc + res.results[4 * b + g]["out"].astype(np.float32)
        outp[b] = acc + bo[None, :]
    kernel.last_result = res
    return outp


# revision 31
# speedup vs baseline: 1.0253x; 1.0253x over previous
"""Entropy-regularized attention (standard MHA fwd) on 8 trn2 cores.

Sharding: core c -> batch b=c//4, head-group g=c%4 (4 of 16 heads).
Each core computes q/k/v for its 256-wide head-group slice, transposed-
layout attention, then a row-split Wo partial product. Host sums the 4
partials per batch and adds bo (the "all-reduce").

v3 restructure vs v2 (261us -> 227us measured):
- Scores are emitted as ROW-TILED HEAD PAIRS: the two heads of an mt
  group have K=64 stationaries at base partitions 0 and 64, so their
  score matmuls auto-derive tile_position (0,0)/(64,0) and execute
  CONCURRENTLY on the PE (each K=64 tile uses half the array rows).
  This halves the PE time of the scores phase (PE busy 231 -> ~191us).
- QG=512 with 4 query groups: per pair-unit the scores for both heads
  land in one [128,2,512] half of the scM PSUM tile and ONE Exp
  (N=1024) covers the pair. PSUM: scM 4 banks + av 2 + ops 2 = 8.
- Softmax 1/l: per qg ONE batched Ln + ONE Exp(-x) on ACT ([97,512]
  costs the same as [1,512]), GPSIMD partition_broadcast fans each
  head's row across 64 partitions (input must sit at PHYSICAL
  partition 0 - it ignores the AP base), DVE tensor_mul into avT.
  The Ln/Exp+muls are release-delayed ~4 units so they queue in ACT's
  FIFO behind already-ready exps. The last qg's second pair instead
  runs per-head Ln/Exp straight from PSUM + PE rb-matmul broadcast to
  minimize the tail after the final exp.
- ALL non-score PE work lives in deferred queues popped under a
  per-unit cost budget (~880ns). av matmuls are release-delayed 3
  units: the PE queue is STRICT FIFO, so an av emitted next to its
  exp blocks later score matmuls while waiting; 3 units late their
  exp has long completed. Readiness gates (qk_ready/v_done +
  ensure_av_through) force-pop so no consumer is ever EMITTED before
  its producer (the Tile framework only orders writer->reader when
  the writer is emitted first; violating that is a silent HW race).
- KNOWN FLOOR: the unit cadence is ~1736ns (vs exp's 1114): with only
  2 scM halves, sc(u) truly depends on exp(u-2), but the Tile
  framework lowers cross-engine deps to engine-frontier counter
  values ("wait ACT-count >= last-emitted-exp"), so whichever of
  sc/exp is emitted later picks up a false one-unit-back edge; every
  emission order tried (fused, split, lookahead-2) leaves one such
  edge on the critical cycle: cadence = exp 1114 + sc chain ~620.
  Breaking it needs >=3 score buffers (PSUM-infeasible at QG=512) or
  raw-bass manual semaphores. TILE_EXHAUSTIVE_MEMORY_SHARE_CHECK=1 is
  set (precise range deps; needed for the manual scM half rotation).
"""

import os
import sys

# Precise (range/bank-exact) memory dependency analysis: without this the
# Tile dep tracker caps its overlap-check work and falls back to
# engine-frontier edges, which serialize the exp stream with the score
# matmuls' PSUM-slot reuse (~600ns per attention unit).
os.environ.setdefault("TILE_EXHAUSTIVE_MEMORY_SHARE_CHECK", "1")

for _p in ("/opt/trn_rl_repo", "/root/.axon_site/_ro/trn_rl_repo"):
    if _p not in sys.path:
        sys.path.insert(0, _p)

import numpy as np

import concourse.bass as bass
import concourse.mybir as mybir
import concourse.tile as tile
from concourse import bacc

P = 128
S = 2048  # sequence length
D = 1024  # hidden
DG = 256  # per-core head-group width (4 heads x 64)
HD = 64
NHL = 4  # heads per core
KT_D = D // P  # 8 contraction tiles for projections
ST = S // P  # 16 sequence tiles
QG = 512  # query-group width
NQG = S // QG  # 4 query groups
NPR = 2  # head pairs per core

F32 = mybir.dt.float32
F32R = mybir.dt.float32r
BF16 = mybir.dt.bfloat16


def build_nc():
    nc = bacc.Bacc(None, target_bir_lowering=False)

    # all inputs pre-arranged on the host into the exact SBUF layouts so
    # every DMA is a contiguous-per-partition blob (large descriptors)
    xT = nc.dram_tensor("xT", [P, 4 * KT_D * 512], BF16, kind="ExternalInput")
    wq = nc.dram_tensor("wq", [P, KT_D * DG], BF16, kind="ExternalInput")
    wk = nc.dram_tensor("wk", [P, KT_D * DG], BF16, kind="ExternalInput")
    wv = nc.dram_tensor("wv", [P, KT_D * DG], BF16, kind="ExternalInput")
    wo = nc.dram_tensor("wo", [P, 2 * D], BF16, kind="ExternalInput")
    bq = nc.dram_tensor("bq", [P, 2], F32, kind="ExternalInput")
    bk = nc.dram_tensor("bk", [P, 2], F32, kind="ExternalInput")
    bv = nc.dram_tensor("bv", [1, DG], F32R, kind="ExternalInput")
    out = nc.dram_tensor("out", [S, D], BF16, kind="ExternalOutput")

    with tile.TileContext(nc) as tc:
        _body(tc, nc, xT, wq, wk, wv, wo, bq, bk, bv, out)

    # Pin Exp/Ln to the one table set holding both: strip them from the
    # competing sets (dict order and size preserved, so act_func_set_id
    # indices stay valid). Without this the table-load pass can bounce
    # between table sets.
    import concourse.bacc as _bacc_mod

    _orig_tables = _bacc_mod.get_activation_tables

    def _pinned_tables(arch):
        t = _orig_tables(arch)
        for name, fns in t.items():
            if name != "natural_log_exp_and_others":
                fns.discard(mybir.ActivationFunctionType.Exp)
                fns.discard(mybir.ActivationFunctionType.Ln)
        return t

    _bacc_mod.get_activation_tables = _pinned_tables
    try:
        nc.compile()
    finally:
        _bacc_mod.get_activation_tables = _orig_tables
    return nc


def _body(tc, nc, xT, wq, wk, wv, wo, bq, bk, bv, out):
    from collections import deque
    from contextlib import ExitStack

    with ExitStack() as ctx:
        ctx.enter_context(
            nc.allow_low_precision(
                reason="bf16 matmul inputs; accumulation is fp32 PSUM"
            )
        )
        persist = ctx.enter_context(tc.tile_pool(name="persist", bufs=1))
        expool = ctx.enter_context(tc.tile_pool(name="expool", bufs=16))
        ulpool = ctx.enter_context(tc.tile_pool(name="ulpool", bufs=12))
        rpool = ctx.enter_context(tc.tile_pool(name="rpool", bufs=4))
        l4pool = ctx.enter_context(tc.tile_pool(name="l4pool", bufs=4))
        r4pool = ctx.enter_context(tc.tile_pool(name="r4pool", bufs=2))
        opool = ctx.enter_context(tc.tile_pool(name="opool", bufs=3))
        # PSUM budget (8 banks): scM [128,4,512]=4 (manually rotated
        # halves), avM [65,2,512]=2, ops 2x[128,512]=2. scM/avM are ONE
        # persistent tile each: pool slot rotation pins a conservative
        # WAR edge to the ACT frontier at emission time (costing ~600ns
        # per unit); manual ranges get precise subtile dependencies.
        ps_persist = ctx.enter_context(
            tc.tile_pool(name="ps_persist", bufs=1, space="PSUM")
        )
        ps_av = ctx.enter_context(tc.tile_pool(name="ps_av", bufs=2, space="PSUM"))
        ps_o = ctx.enter_context(tc.tile_pool(name="ps_o", bufs=2, space="PSUM"))

        qT_sb = persist.tile([P, 2, S], BF16)
        kT_sb = persist.tile([P, 2, S], BF16)
        v_sb = persist.tile([P, ST, NHL * 65], BF16)  # 65-striped: col 64 = ones
        avT = [
            persist.tile([P, 2, QG], BF16, tag=f"avT{g}", name=f"avT{g}")
            for g in range(NQG)
        ]
        wo_sb = persist.tile([P, 2, D], BF16)
        ones_row = persist.tile([1, P], F32R)
        xT_sb = persist.tile([P, 4, KT_D, 512], BF16)
        wq_sb = persist.tile([P, KT_D, DG], BF16, tag="wq")
        wk_sb = persist.tile([P, KT_D, DG], BF16, tag="wk")
        wv_sb = persist.tile([P, KT_D, DG], BF16, tag="wv")
        bq_sb = persist.tile([P, 2], F32, tag="bq")
        bk_sb = persist.tile([P, 2], F32, tag="bk")
        bv_sb = persist.tile([1, DG], F32R, tag="bv")

        # DMAs ordered to match first-use: wk then the first xT column
        # block gate the first projection chain.
        xTr = xT.rearrange("p (cb kt s) -> p cb kt s", kt=KT_D, s=512)
        nc.sync.dma_start(wk_sb[:], wk.rearrange("p (kt n) -> p kt n", n=DG))
        nc.sync.dma_start(xT_sb[:, 0, 0:2], xTr[:, 0, 0:2])
        nc.sync.dma_start(xT_sb[:, 0, 2:4], xTr[:, 0, 2:4])
        nc.sync.dma_start(bk_sb[:], bk[:])
        nc.sync.dma_start(wq_sb[:], wq.rearrange("p (kt n) -> p kt n", n=DG))
        nc.sync.dma_start(xT_sb[:, 0, 4:6], xTr[:, 0, 4:6])
        nc.sync.dma_start(xT_sb[:, 0, 6:8], xTr[:, 0, 6:8])
        nc.sync.dma_start(bq_sb[:], bq[:])
        nc.sync.dma_start(wv_sb[:], wv.rearrange("p (kt n) -> p kt n", n=DG))
        nc.sync.dma_start(bv_sb[:], bv[:])
        nc.sync.dma_start(xT_sb[:, 1], xTr[:, 1])
        nc.sync.dma_start(xT_sb[:, 2], xTr[:, 2])
        nc.sync.dma_start(xT_sb[:, 3], xTr[:, 3])
        nc.sync.dma_start(wo_sb[:], wo.rearrange("p (kt n) -> p kt n", n=D))

        # memset can't emit float32r; stage fp32 ones and copy-cast
        ones_f32 = persist.tile([P, P], F32)
        nc.vector.memset(ones_f32[:], 1.0)
        nc.vector.tensor_copy(ones_row[:], ones_f32[0:1, :])
        ones_all = persist.tile([P, P], BF16)
        nc.vector.tensor_copy(ones_all[:], ones_f32[:])
        nc.vector.tensor_copy(
            v_sb.rearrange("p st (h w) -> p st h w", w=65)[:, :, :, 64],
            ones_f32[:, 0:64].rearrange("p (st h) -> p st h", h=4),
        )

        # ---- projection building blocks ----
        def qk_half(ps, wsb, mt, nq, kts):
            for kt in kts:
                nc.tensor.matmul(
                    ps[:, 0:512],
                    wsb[:, kt, mt * P : (mt + 1) * P],
                    xT_sb[:, nq, kt, :],
                    start=(kt == 0),
                    stop=(kt == KT_D - 1),
                )

        def qk_evict(ps, bsb, dest, mt, nq):
            nc.vector.tensor_scalar_add(
                dest[:, mt, nq * 512 : (nq + 1) * 512],
                ps[:, 0:512],
                bsb[:, mt : mt + 1],
            )

        # emission-order readiness flags: a score matmul may only be
        # EMITTED after the projection chain writing its qT/kT region has
        # been emitted (the Tile framework tracks writer->reader deps in
        # program order; a later-emitted writer would be a race).
        qk_ready = {}  # ("q"/"k", mt, nq) -> True

        def qk_full(wsb, bsb, dest, mt, nq, key):
            ps = ps_o.tile([P, 512], F32, tag="ops", name="qkps")
            qk_half(ps, wsb, mt, nq, range(0, KT_D))
            qk_evict(ps, bsb, dest, mt, nq)
            qk_ready[(key, mt, nq)] = True

        def qk_pieces(wsb, bsb, dest, mt, nq, key, pieces=4):
            # split the 8-matmul K-chain into `pieces` closures; the last
            # one carries the bias eviction. cost ~= (8/pieces)*213ns
            box = []
            step = KT_D // pieces

            def make(pi):
                def run():
                    if pi == 0:
                        box.append(
                            ps_o.tile([P, 512], F32, tag="ops", name="qkps")
                        )
                    ps = box[-1]
                    qk_half(ps, wsb, mt, nq, range(pi * step, (pi + 1) * step))
                    if pi == pieces - 1:
                        qk_evict(ps, bsb, dest, mt, nq)
                        box.pop()
                        qk_ready[(key, mt, nq)] = True

                return run

            return [(step * 215, make(pi)) for pi in range(pieces)]

        v_done = [False] * ST

        def v_group_half(st, half):
            # half 0: kts 0-3; half 1: kts 4-7 + bias + evict
            def run():
                if half == 0:
                    v_group_half.box[st] = ps_o.tile(
                        [P, 512], F32, tag="ops", name="vps"
                    )
                    ps = v_group_half.box[st]
                    for kt in range(0, 4):
                        nc.tensor.matmul(
                            ps[:, 0:DG],
                            xT_sb[:, st // 4, kt, (st % 4) * P : (st % 4 + 1) * P],
                            wv_sb[:, kt, :],
                            start=(kt == 0),
                            stop=False,
                        )
                else:
                    ps = v_group_half.box.pop(st)
                    for kt in range(4, KT_D):
                        nc.tensor.matmul(
                            ps[:, 0:DG],
                            xT_sb[:, st // 4, kt, (st % 4) * P : (st % 4 + 1) * P],
                            wv_sb[:, kt, :],
                            start=False,
                            stop=False,
                        )
                    nc.tensor.matmul(
                        ps[:, 0:DG],
                        ones_row[0:1, 0:P],
                        bv_sb[0:1, :],
                        start=False,
                        stop=True,
                    )
                    nc.vector.tensor_copy(
                        v_sb.rearrange("p st (h w) -> p st h w", w=65)[
                            :, st, :, 0:64
                        ],
                        ps[:, 0:DG].rearrange("p (h w) -> p h w", w=64),
                    )
                    v_done[st] = True

            return run

        v_group_half.box = {}

        def v_group_full(st):
            v_group_half(st, 0)()
            v_group_half(st, 1)()

        # ---- deferred machinery ----
        # The PE engine queue is strict FIFO: an emitted matmul that waits
        # on a semaphore blocks everything emitted after it. So av pieces
        # (which wait on their exp) are RELEASE-DELAYED ~3 units: by the
        # time they enter the queue their exp has long finished, and the
        # next units' score matmuls are never stuck behind them. `delayed`
        # is strictly FIFO (entries release in order once their
        # release-iter arrives and their v-group gate is satisfied).
        # avq: released attention follow-up (av/norms), popped first.
        # defq: projection / Wo filler under a per-unit cost budget.
        avq = deque()
        defq = deque()
        delayed = deque()  # (release_iter, gate_st_or_None, kind, cost, fn)

        BUDGET = 880
        n_av_emitted = [0]  # count of av matmul pieces emitted (2 per unit)

        def release_delayed(i):
            while delayed and delayed[0][0] <= i:
                _, gate, kind, cost, fn = delayed[0]
                if gate is not None and not v_done[gate]:
                    break
                delayed.popleft()
                avq.append((kind, cost, fn))

        def pop_one_av():
            if avq:
                kind, _, fn = avq.popleft()
                fn()
                if kind == "av":
                    n_av_emitted[0] += 1
                return True
            return False

        def pop_one_def():
            if defq:
                _, fn = defq.popleft()
                fn()
                return True
            return False

        def pop_pieces(i):
            spent = 0
            release_delayed(i)
            while avq and spent < BUDGET:
                kind, cost, fn = avq.popleft()
                fn()
                if kind == "av":
                    n_av_emitted[0] += 1
                spent += cost
            while defq and spent < BUDGET:
                cost, fn = defq.popleft()
                fn()
                spent += cost

        def ensure_qk(key, mt, nq):
            # force-pop until the projection chain for this region has run
            while not qk_ready.get((key, mt, nq)):
                if not pop_one_def():
                    raise RuntimeError(f"deadlock: {key} mt{mt} nq{nq}")

        def ensure_av_through(unit_idx):
            # all av pieces of units <= unit_idx emitted (ex-slot reuse)
            while n_av_emitted[0] < 2 * (unit_idx + 1):
                if pop_one_av():
                    continue
                if delayed:
                    _, gate, kind, cost, fn = delayed[0]
                    if gate is None or v_done[gate]:
                        delayed.popleft()
                        avq.append((kind, cost, fn))
                        continue
                if not pop_one_def():
                    raise RuntimeError("deadlock: av drain")

        # ---- attention phase ----
        EXBUFS = 16  # must match expool bufs

        scM = ps_persist.tile([P, 4, 512], F32, tag="scM", name="scM")

        def sc_emit(u):
            # score pair for unit u into scM half u%2. Emitted BEFORE
            # exp(u-1) (see the unit loop), so the conservative
            # PSUM-write-after-ACT-read frontier edge the framework pins
            # on this matmul is exp(u-2) -- exactly the true dependency
            # (exp(u-2) is the reader of this half). The pair then
            # pre-executes during exp(u-1) and never gates the exp stream.
            pr, qg, kt = units[u]
            ensure_qk("k", pr, kt // 4)
            ensure_qk("q", pr, qg)
            mt, q0 = pr, qg * QG
            half = (u % 2) * 2
            for hh in range(2):
                po = hh * 64
                nc.tensor.matmul(
                    scM[:, half + hh, :],
                    kT_sb[po : po + 64, mt, kt * P : (kt + 1) * P],
                    qT_sb[po : po + 64, mt, q0 : q0 + 512],
                    start=True,
                    stop=True,
                )

        def exp_emit(u):
            half = (u % 2) * 2
            ex = expool.tile([P, 2, 512], BF16, tag="ex", name="ex")
            nc.scalar.activation(
                ex[:], scM[:, half : half + 2, :],
                mybir.ActivationFunctionType.Exp, scale=0.125,
            )
            return ex

        avps = {}  # hh -> live av psum tile for current (pr, qg)

        def av_piece(pr, qg, kt, hh, ex):
            h = 2 * pr + hh

            def run():
                if kt == 0:
                    avps[hh] = ps_av.tile([65, 512], F32, tag="av", name="av")
                nc.tensor.matmul(
                    avps[hh][0:65, :],
                    v_sb[:, kt, h * 65 : h * 65 + 65],
                    ex[:, hh, :],
                    start=(kt == 0),
                    stop=(kt == ST - 1),
                )

            return (215, run)

        # Softmax normalization: per (pr,qg,head) the av PSUM is copied to
        # SBUF (ul) and its denominator row gathered into l4[qg] at
        # partition 32h. Once a qg's heads are all gathered, ONE batched
        # Ln + ONE Exp(-x) on ACT produce 1/l ([97,512] costs the same as
        # [1,512]); GPSIMD broadcasts each head's row across 64 partitions
        # and DVE multiplies into avT. The Ln/Exp+muls are DELAYED ~4
        # units so they land in ACT's FIFO behind already-ready exps
        # (emitting them immediately stalls the exp stream on the av->ul->
        # gather dependency chain). The LAST qg's second pair instead runs
        # per-head Ln/Exp straight from PSUM with a PE-matmul broadcast,
        # minimizing the serial tail after the final exp.
        uls = {}
        l4s = {}
        r4box = {}

        def p_ul_make(pr, qg, hh, skip_l=False):
            h = 2 * pr + hh

            def p_ul():
                av = avps.pop(hh)
                if qg not in l4s:
                    l4s[qg] = l4pool.tile([97, 512], F32, tag="l4", name="l4")
                    nc.gpsimd.memset(l4s[qg][:], 1.0)
                ul = ulpool.tile([65, 512], F32, tag="ul", name="ul")
                nc.vector.tensor_copy(ul[:], av[0:65, :])
                if not skip_l:
                    nc.vector.tensor_copy(
                        l4s[qg][32 * h : 32 * h + 1, :], av[64:65, :]
                    )
                uls[(qg, h)] = ul

            return p_ul

        def p_norm_make(qg, hi):
            def p_norm():
                # 1/l = exp(-ln(l)) for heads 0..hi/32 in one Ln + one Exp
                l4 = l4s.pop(qg)
                ls = r4pool.tile([97, 512], F32, tag="ls", name="ls")
                nc.scalar.activation(
                    ls[0:hi], l4[0:hi], mybir.ActivationFunctionType.Ln
                )
                r4 = r4pool.tile([97, 512], F32, tag="r4", name="r4")
                nc.scalar.activation(
                    r4[0:hi], ls[0:hi],
                    mybir.ActivationFunctionType.Exp, scale=-1.0,
                )
                r4box[qg] = r4

            return p_norm

        def p_mul_make(qg, h2, unlock_wo=False):
            def p_mul():
                mt2, po2 = h2 // 2, (h2 % 2) * 64
                ul = uls.pop((qg, h2))
                # partition_broadcast reads physical partition 0: stage
                # this head's reciprocal row to a base-0 tile first
                r1 = rpool.tile([1, 512], F32, tag="r1", name="r1")
                nc.vector.tensor_copy(
                    r1[:], r4box[qg][32 * h2 : 32 * h2 + 1, :]
                )
                rbb = rpool.tile([64, 512], F32, tag="rbb", name="rbb")
                nc.gpsimd.partition_broadcast(rbb[:], r1[:], channels=64)
                nc.vector.tensor_mul(
                    out=avT[qg][po2 : po2 + 64, mt2, :],
                    in0=ul[0:64, :],
                    in1=rbb[:],
                )
                if unlock_wo:
                    defq.extend(wo_pieces(qg))

            return p_mul

        def tail_head_pieces(qg, hh):
            # last qg, second pair: Ln/Exp straight from the av PSUM, PE
            # rb-matmul broadcast (PE is idle in the tail), DVE multiply.
            h = 2 + hh
            box = {}

            def p_lnexp():
                av = avps[hh]
                lnr = rpool.tile([1, 512], F32, tag="lnr", name="lnr")
                nc.scalar.activation(
                    lnr[:], av[64:65, :], mybir.ActivationFunctionType.Ln
                )
                r1 = rpool.tile([1, 512], BF16, tag="r1b", name="r1b")
                nc.scalar.activation(
                    r1[:], lnr[:], mybir.ActivationFunctionType.Exp,
                    scale=-1.0,
                )
                box["r1"] = r1

            def p_ul():
                av = avps.pop(hh)
                ul = ulpool.tile([65, 512], F32, tag="ul", name="ul")
                nc.vector.tensor_copy(ul[0:64, :], av[0:64, :])
                box["ul"] = ul

            def p_rbmul():
                rb = ps_o.tile([P, 512], F32, tag="ops", name="rb")
                nc.tensor.matmul(
                    rb[0:64, :],
                    ones_all[0:1, 0:64],
                    box["r1"][:],
                    start=True,
                    stop=True,
                )
                nc.vector.tensor_mul(
                    out=avT[qg][hh * 64 : hh * 64 + 64, 1, :],
                    in0=box["ul"][0:64, :],
                    in1=rb[0:64, :],
                )
                if hh == 1:
                    defq.extend(wo_pieces(qg))

            return [(50, p_lnexp), (50, p_ul), (250, p_rbmul)]

        ot_box = {}

        def wo_pieces(qg):
            # per (sti, nd): 2 matmuls (kt2 accumulation) + eviction; the
            # ops-pool slot is alloc'd and freed within one piece so the
            # bufs=2 rotation can interleave with qk/v pieces.
            pieces = []

            def make(sti, nd):
                def run():
                    st = qg * (QG // P) + sti
                    if nd == 0:
                        ot_box[sti] = opool.tile([P, D], BF16, tag="ot", name="ot")
                    ot = ot_box[sti]
                    pp = ps_o.tile([P, 512], F32, tag="ops", name="pp")
                    for kt2 in range(2):
                        nc.tensor.matmul(
                            pp[:],
                            avT[qg][:, kt2, sti * P : (sti + 1) * P],
                            wo_sb[:, kt2, nd * 512 : (nd + 1) * 512],
                            start=(kt2 == 0),
                            stop=(kt2 == 1),
                        )
                    nc.vector.tensor_copy(ot[:, nd * 512 : (nd + 1) * 512], pp[:])
                    if nd == 1:
                        del ot_box[sti]
                        nc.sync.dma_start(out[st * P : (st + 1) * P, :], ot[:])

                return run

            for sti in range(QG // P):
                for nd in range(2):
                    pieces.append((460, make(sti, nd)))
            return pieces

        # ---- upfront phase: only what the FIRST sc/exp needs (k+q mt0
        # nq0); v groups are deferred (av lags behind the exp stream) ----
        qk_full(wk_sb, bk_sb, kT_sb, 0, 0, "k")
        qk_full(wq_sb, bq_sb, qT_sb, 0, 0, "q")

        # ---- static filler: ordered by need-by unit ----
        # kT mt0 fully by unit ~12 (sc consumes kt blocks 4/8/12 at units
        # 4/8/12); q(mt0,nq1..3) by units 16/32/48; v(st) before av(st)
        # emission (forced by ensure_av_through); mt1 chains by unit 64.
        for st in (0, 1):
            defq.append((500, v_group_half(st, 0)))
            defq.append((560, v_group_half(st, 1)))
        defq.extend(qk_pieces(wk_sb, bk_sb, kT_sb, 0, 1, "k"))
        defq.extend(qk_pieces(wk_sb, bk_sb, kT_sb, 0, 2, "k"))
        defq.extend(qk_pieces(wk_sb, bk_sb, kT_sb, 0, 3, "k"))
        defq.extend(qk_pieces(wq_sb, bq_sb, qT_sb, 0, 1, "q"))
        for st in (2, 3, 4, 5):
            defq.append((500, v_group_half(st, 0)))
            defq.append((560, v_group_half(st, 1)))
        defq.extend(qk_pieces(wq_sb, bq_sb, qT_sb, 0, 2, "q"))
        for st in (6, 7, 8, 9):
            defq.append((500, v_group_half(st, 0)))
            defq.append((560, v_group_half(st, 1)))
        defq.extend(qk_pieces(wq_sb, bq_sb, qT_sb, 0, 3, "q"))
        for st in (10, 11, 12, 13, 14, 15):
            defq.append((500, v_group_half(st, 0)))
            defq.append((560, v_group_half(st, 1)))
        for nq in range(4):
            defq.extend(qk_pieces(wk_sb, bk_sb, kT_sb, 1, nq, "k"))
            defq.extend(qk_pieces(wq_sb, bq_sb, qT_sb, 1, nq, "q"))

        # ---- the unit loop ----
        units = [
            (pr, qg, kt)
            for pr in range(NPR)
            for qg in range(NQG)
            for kt in range(ST)
        ]
        sc_emit(0)
        sc_emit(1)
        for i, (pr, qg, kt) in enumerate(units):
            ensure_av_through(i - EXBUFS)
            ex = exp_emit(i)
            if i + 2 < len(units):
                sc_emit(i + 2)
            for hh in range(2):
                delayed.append((i + 3, kt, "av", *av_piece(pr, qg, kt, hh, ex)))
            if kt == ST - 1:
                last_qg = qg == NQG - 1
                if pr == 0:
                    for hh in range(2):
                        delayed.append(
                            (i + 4, None, "norm", 50, p_ul_make(pr, qg, hh))
                        )
                    if last_qg:
                        # qg3 pair-0 heads: batched 2-head norm
                        delayed.append(
                            (i + 5, None, "norm", 50, p_norm_make(qg, 33))
                        )
                        delayed.append(
                            (i + 5, None, "norm", 50, p_mul_make(qg, 0))
                        )
                        delayed.append(
                            (i + 5, None, "norm", 50, p_mul_make(qg, 1))
                        )
                elif not last_qg:
                    for hh in range(2):
                        delayed.append(
                            (i + 4, None, "norm", 50, p_ul_make(pr, qg, hh))
                        )
                    delayed.append(
                        (i + 5, None, "norm", 50, p_norm_make(qg, 97))
                    )
                    for h2 in range(4):
                        delayed.append(
                            (i + 5, None, "norm", 50,
                             p_mul_make(qg, h2, unlock_wo=(h2 == 3)))
                        )
                else:
                    # last unit: minimal-latency tail for heads 2,3
                    t0 = tail_head_pieces(qg, 0)
                    t1 = tail_head_pieces(qg, 1)
                    for piece in (t0[0], t1[0], t0[1], t1[1], t0[2], t1[2]):
                        delayed.append((i, None, "norm", *piece))
            pop_pieces(i)
        # ---- tail: drain everything ----
        while delayed or avq or defq:
            release_delayed(10 ** 9)
            if pop_one_av():
                continue
            if not pop_one_def():
                if delayed:
                    raise RuntimeError("tail deadlock")


_NC_CACHE = None


def get_nc():
    global _NC_CACHE
    if _NC_CACHE is None:
        _NC_CACHE = build_nc()
    return _NC_CACHE


def make_in_maps(x, Wq, bq, Wk, bk, Wv, bv, Wo, bo):
    import ml_dtypes

    bf16 = ml_dtypes.bfloat16

    def w_arr(W, sl):
        # [D, DG] -> [p, kt*DG]: W[kt*128+p, n] at [p, kt, n]
        return np.ascontiguousarray(
            W[:, sl].reshape(KT_D, P, DG).transpose(1, 0, 2).reshape(P, -1)
        ).astype(bf16)

    in_maps = []
    for c in range(8):
        b, g = c // 4, c % 4
        sl = slice(g * DG, (g + 1) * DG)
        # x[b].T is [D, S]; SBUF wants [p, cb, kt, 512] with row kt*128+p,
        # col cb*512+s
        xt = (
            x[b]
            .T.reshape(KT_D, P, 4, 512)
            .transpose(1, 2, 0, 3)
            .reshape(P, -1)
            .astype(bf16)
        )
        wo_a = np.ascontiguousarray(
            Wo[sl, :].reshape(2, P, D).transpose(1, 0, 2).reshape(P, -1)
        ).astype(bf16)
        in_maps.append(
            {
                "xT": np.ascontiguousarray(xt),
                "wq": w_arr(Wq, sl),
                "wk": w_arr(Wk, sl),
                "wv": w_arr(Wv, sl),
                "wo": wo_a,
                "bq": np.ascontiguousarray(bq[sl].reshape(2, P).T),
                "bk": np.ascontiguousarray(bk[sl].reshape(2, P).T),
                "bv": np.ascontiguousarray(bv[sl].reshape(1, DG)),
            }
        )
    return in_maps


def kernel(x, Wq, bq, Wk, bk, Wv, bv, Wo, bo, _run_kwargs=None):
    from concourse.bass_utils import run_bass_kernel_spmd

    x = np.asarray(x, dtype=np.float32)
    nc = get_nc()
    in_maps = make_in_maps(
        x,
        np.asarray(Wq, np.float32),
        np.asarray(bq, np.float32),
        np.asarray(Wk, np.float32),
        np.asarray(bk, np.float32),
        np.asarray(Wv, np.float32),
        np.asarray(bv, np.float32),
        np.asarray(Wo, np.float32),
        np.asarray(bo, np.float32),
    )
    res = run_bass_kernel_spmd(
        nc, in_maps, core_ids=list(range(8)), **(_run_kwargs or {})
    )
    bo = np.asarray(bo, np.float32)
    outp = np.empty((2, S, D), dtype=np.float32)
    for b in range(2):
        acc = res.results[4 * b]["out"].astype(np.float32)
        for g in range(1, 4):
            acc = acc + res.results[4 * b + g]["out"].astype(np.float32)
        outp[b] = acc + bo[None, :]
    kernel.last_result = res
    return outp
